# revision 37
# baseline (speedup 1.0000x reference)
"""Trainium2 Bass kernel for causal ReLU attention (no softmax).

  qkv = x @ W.T + b;  per head: s = (q k^T) * 1/sqrt(64)
  p = relu(causal(s));  y = p @ v

Sharding: 8 cores = 2 batches x 4 head-groups (3 heads each). Each core:
  - qk-projection computed transposed (features on partitions) so q/k land
    as qT/kT [64, T] ready to be matmul operands with d on partitions
  - v-projection computed natural [T, 192]
  - scores: K=128 zero-padded contraction (two heads share a 128-partition
    tile; lhsT = [kT_h; 0] makes each head's matmul full-width)
  - block-causal: fully-masked key blocks skipped, diagonal blocks get a
    restricted column range + triangle mask via one fused DVE/Pool op
  - two chains (A, B) software-pipelined on the PE queue with one-step
    lookahead so the PSUM->SBUF relu never stalls the PE; relu work is
    spread across DVE (A), ACT (B full) and Pool (B diag); y PSUM
    evacuation runs on Pool.
All matmul operands fp16 (fp32 PSUM accumulation). Host does the
shard/transpose/cast prep and the final gather (pure numpy).
"""
import numpy as np

import concourse.bass as bass
import concourse.mybir as mybir
import concourse.tile as tile
from concourse import bacc
from concourse.bass_utils import run_bass_kernel_spmd

F32 = mybir.dt.float32
F16 = mybir.dt.float16

B, T, C = 2, 2048, 768
NH = 12          # total heads
HPC = 3          # heads per core
D = 64
NCORES = 8
CC = 6           # contraction chunks (768 / 128)
TB = 512         # query block
KB = 128         # key block
NTB = T // TB    # 4
NKB = T // KB    # 16

Relu = mybir.ActivationFunctionType.Relu
Copy = mybir.ActivationFunctionType.Identity
MAX = mybir.AluOpType.max
MULT = mybir.AluOpType.mult


def _build(reps=1, stage=4):
    nc = bacc.Bacc(None, target_bir_lowering=False, debug=False)
    xT = nc.declare_dram_parameter("xT", [C, T], F16, isOutput=False)
    wqk = nc.declare_dram_parameter("wqk", [C, 384], F16, isOutput=False)
    wv = nc.declare_dram_parameter("wv", [C, 192], F16, isOutput=False)
    bias_qk = nc.declare_dram_parameter("bias_qk", [3, 128], F32, isOutput=False)
    scale_qk = nc.declare_dram_parameter("scale_qk", [3, 128], F32, isOutput=False)
    bias_v = nc.declare_dram_parameter("bias_v", [128, 192], F32, isOutput=False)
    yt_out = nc.declare_dram_parameter("yt", [HPC, D, T], F32, isOutput=True)

    with tile.TileContext(nc) as tc:
        with tc.tile_pool(name="const", bufs=1) as const, \
             tc.tile_pool(name="xr", bufs=12) as xr, \
             tc.tile_pool(name="vt", bufs=16) as vtp, \
             tc.tile_pool(name="pt", bufs=8) as ptp, \
             tc.tile_pool(name="ys", bufs=4) as ysp, \
             tc.tile_pool(name="psy", bufs=4, space="PSUM") as psy, \
             tc.tile_pool(name="pssc", bufs=4, space="PSUM") as pssc:

            # ---------------- constants ----------------
            bias_sb = const.tile([128, 3], F32)
            scale_sb = const.tile([128, 3], F32)
            nc.sync.dma_start(out=bias_sb, in_=bias_qk[:, :].rearrange("a p -> p a"))
            nc.sync.dma_start(out=scale_sb, in_=scale_qk[:, :].rearrange("a p -> p a"))
            biasv_sb = const.tile([128, 192], F32)
            nc.sync.dma_start(out=biasv_sb, in_=bias_v[:, :])
            # triangle mask M[kk, qq] = 1 if qq >= kk else 0  (f32: faster DVE read)
            mask_sb = const.tile([128, TB], F32)
            nc.vector.memset(mask_sb, 1.0)
            nc.gpsimd.affine_select(
                out=mask_sb, in_=mask_sb,
                compare_op=mybir.AluOpType.is_ge, fill=0.0, base=0,
                pattern=[[1, TB]], channel_multiplier=-1)

            # weight chunks
            wqk_sb = [const.tile([128, 384], F16, tag="wqk", bufs=CC, name=f"wqk{c}") for c in range(CC)]
            wv_sb = [const.tile([128, 192], F16, tag="wv", bufs=CC, name=f"wv{c}") for c in range(CC)]
            for c in range(CC):
                nc.sync.dma_start(out=wqk_sb[c], in_=wqk[c * 128:(c + 1) * 128, :])
                nc.sync.dma_start(out=wv_sb[c], in_=wv[c * 128:(c + 1) * 128, :])

            # persistent attention operand tiles (scores use K=64 contraction
            # with matching base partitions -- no zero padding needed)
            qq01 = const.tile([128, T], F16)   # [qT_h0; qT_h1]
            kAB = const.tile([128, T], F16)    # [kT_h0; kT_h1]
            m2 = const.tile([128, T], F16)     # [kT_h2; qT_h2]
            q2lo = const.tile([64, T], F16)    # qT_h2 shifted to partitions 0-63

            def body():
                # stage: 1=dma only, 2=+proj, 3=+scores/relu, 4=full
                # ---------------- load xT (fp16, pre-cast on host) ---------
                xt = [xr.tile([128, T], F16, tag="xt", name=f"xt{c}") for c in range(CC)]
                for c in range(CC):
                    nc.sync.dma_start(out=xt[c], in_=xT[c * 128:(c + 1) * 128, :])

                if stage < 2:
                    return
                # ---------------- qk projection (transposed) ---------------
                # f-tiles: 0 = [q0; q1], 1 = [k0; k1], 2 = [k2; q2]
                # c-outer order: all 4 query blocks accumulate in parallel
                # (two 2-bank PSUM tiles), so the 4 matmuls sharing one
                # weight chunk are consecutive -> redundant Ldweights are
                # stripped post-finalize by _dedup_ldweights.
                for ft in range(3):
                    pst = [pssc.tile([128, TB], F32, tag="s",
                                     name=f"pj{ft}_{i}") for i in range(NTB)]
                    for c in range(CC):
                        for tb in range(NTB):
                            nc.tensor.matmul(
                                pst[tb],
                                wqk_sb[c][:, ft * 128:(ft + 1) * 128],
                                xt[c][:, tb * TB:(tb + 1) * TB],
                                start=(c == 0), stop=(c == CC - 1))
                    dst = (qq01, kAB, m2)[ft]
                    for i in range(NTB):
                        nc.scalar.activation(
                            dst[:, i * TB:(i + 1) * TB], pst[i], Copy,
                            bias=bias_sb[:, ft:ft + 1],
                            scale=scale_sb[:, ft:ft + 1])
                # shift qT_h2 to partitions 0-63 (SBUF->SBUF DMA)
                nc.sync.dma_start(out=q2lo[:, :], in_=m2[64:128, :])

                # ---------------- v projection (natural layout) ------------
                v_sb = []
                for tt in range(NKB):
                    ps = pssc.tile([128, 192], F32, tag="s")
                    for c in range(CC):
                        nc.tensor.matmul(
                            ps, xt[c][:, tt * 128:(tt + 1) * 128], wv_sb[c],
                            start=(c == 0), stop=(c == CC - 1))
                    vt = vtp.tile([128, 192], F16, tag="v")
                    nc.vector.tensor_add(vt, ps, biasv_sb)
                    v_sb.append(vt)

                if stage < 3:
                    return
                # ---------------- attention ----------------
                # per head: (kT rows, qT rows), both 64-partition slices with
                # matching base partitions (K=64 contraction)
                heads = [(kAB[0:64, :], qq01[0:64, :]),
                         (kAB[64:128, :], qq01[64:128, :]),
                         (m2[0:64, :], q2lo[:, :])]

                def relu_op(eng, out, in_, masked, n):
                    # out = relu(in_) [* mask] on the given engine.
                    # Pool/GPSIMD cannot read PSUM, so masked relus run either
                    # fully on DVE (fused max*mask) or as ACT relu followed by
                    # an in-place Pool affine_select on the SBUF result.
                    if masked:
                        if eng == "dve":
                            nc.vector.scalar_tensor_tensor(
                                out=out, in0=in_, scalar=0.0,
                                in1=mask_sb[:, 0:n], op0=MAX, op1=MULT)
                        else:
                            nc.scalar.activation(out, in_, Relu)
                            nc.gpsimd.affine_select(
                                out=out, in_=out,
                                compare_op=mybir.AluOpType.is_ge, fill=0.0,
                                base=0, pattern=[[1, n]],
                                channel_multiplier=-1)
                    elif eng == "act":
                        nc.scalar.activation(out, in_, Relu)
                    else:
                        nc.vector.tensor_scalar(out, in_, 0.0, None, MAX)

                def make_units(hl, qb, ytp_f, side):
                    """Build the per-key-block unit list for one chain.

                    One unit = one key block: a single score matmul, a relu,
                    and a single y matmul accumulating into this chain's own
                    [64, TB] PSUM bank (tile_position (0,0) for every chain,
                    so same-head pairs share their y weight loads).
                    """
                    kz, qq = heads[hl]
                    nkb = 4 * qb + 4
                    nfull = 4 * qb
                    uid = f"{side}{hl}_{qb}"
                    units = []
                    box = {}

                    def mk(kb):
                        diag = kb >= nfull
                        lo = (kb - nfull) * KB if diag else 0

                        def emit_s():
                            sp = pssc.tile([128, TB], F32, tag="s",
                                           name=f"sp{uid}_{kb}")
                            box[kb] = sp
                            nc.tensor.matmul(
                                sp[:, lo:TB],
                                kz[:, kb * KB:(kb + 1) * KB],
                                qq[:, qb * TB + lo:(qb + 1) * TB],
                                start=True, stop=True)

                        def emit_relu(eng):
                            sp = box.pop(kb)
                            pt = ptp.tile([128, TB], F16, tag="p2",
                                          name=f"pt{uid}_{kb}")
                            box[("p", kb)] = pt
                            relu_op(eng, pt[:, lo:TB], sp[:, lo:TB],
                                    diag, TB - lo)

                        def emit_y():
                            pt = box.pop(("p", kb))
                            if stage >= 4:
                                nc.tensor.matmul(
                                    ytp_f()[:, lo:TB],
                                    v_sb[kb][:, hl * 64:(hl + 1) * 64],
                                    pt[:, lo:TB],
                                    start=(kb == 0), stop=(kb == nkb - 1))
                        return (emit_s, emit_relu, emit_y), diag

                    for kb in range(nkb):
                        units.append(mk(kb))

                    def emit_out():
                        if stage >= 4:
                            ys = ysp.tile([64, TB], F32, tag="ys",
                                          name=f"ys{uid}")
                            nc.scalar.activation(ys, ytp_f(), Copy)
                            nc.sync.dma_start(
                                out=yt_out[hl, :, qb * TB:(qb + 1) * TB],
                                in_=ys)
                    return units, emit_out

                # One global software-pipelined stream over per-key-block
                # steps. Same-head pairs: at step k both sides consume key
                # block k, so the score Ldweights is shared, and because both
                # sides' y accumulators sit at partitions 0:64 of their own
                # banks (tile_position (0,0)), the y Ldweights is shared too
                # (_dedup_ldweights strips the duplicates).
                pairs = [((0, 0), (0, 1)), ((1, 0), (1, 1)), ((2, 0), (2, 1)),
                         ((0, 2), (0, 3)), ((1, 2), (1, 3)), ((2, 2), (2, 3))]
                steps = []
                ytp_cache = {}

                def mk_ytp(pi, side):
                    def f():
                        key = (pi, side)
                        if key not in ytp_cache:
                            ytp_cache[key] = psy.tile(
                                [64, TB], F32, tag="y", name=f"yt_{pi}{side}")
                        return ytp_cache[key]
                    return f

                for pi, (ca, cb) in enumerate(pairs):
                    usA, outA = make_units(ca[0], ca[1], mk_ytp(pi, "a"),
                                           f"a{pi}")
                    usB, outB = make_units(cb[0], cb[1], mk_ytp(pi, "b"),
                                           f"b{pi}")
                    nA, nB = len(usA), len(usB)
                    for k in range(max(nA, nB)):
                        st = {}
                        for side, us, out, cn in (("a", usA, outA, nA),
                                                  ("b", usB, outB, nB)):
                            if k < len(us):
                                (es, er, ey), diag = us[k]
                                st[side] = (es, er, ey, diag,
                                            out if k == cn - 1 else None)
                        steps.append(st)

                def emit(i, phase):
                    # engine choice by global step parity: A and B always land
                    # on different engines; a masked relu assigned to ACT runs
                    # as ACT relu + Pool affine_select (relu_op handles it).
                    st = steps[i]
                    for side in ("a", "b"):
                        if side not in st:
                            continue
                        es, er, ey, diag, out = st[side]
                        if phase == 0:
                            es()
                        elif phase == 1:
                            if side == "a":
                                eng = ("dve", "act")[i % 2]
                            else:
                                eng = ("act", "dve")[i % 2]
                            er(eng)
                        else:
                            ey()
                            if out is not None:
                                out()

                n = len(steps)
                emit(0, 0)
                for k in range(n):
                    if k + 1 < n:
                        emit(k + 1, 0)     # lookahead scores
                    emit(k, 1)             # relu of step k
                    emit(k, 2)             # y of step k (+chain output)

            if reps == 1:
                body()
            elif reps < 0:
                with tc.For_i(0, -reps, 1):
                    body()
            else:
                for _ in range(reps):
                    body()

    nc.finalize()
    _dedup_ldweights(nc)
    return nc


def _dedup_ldweights(nc):
    """Drop Ldweights that reload the exact weights already resident.

    The legalizer pairs every Matmult with its own Ldweights; matmuls
    emitted back-to-back with the same stationary operand reload it
    needlessly (~53-106ns each on HW). Safe removal requires: identical
    weights AP and tile config, nothing but matmuls/event-sems between
    the two loads, and no semaphore waits/updates on the removed load.
    """
    def key(inst):
        return (repr(inst.ins[0]), repr(inst.tile_size),
                repr(inst.tile_position), repr(inst.perf_mode),
                repr(inst.is_transpose))

    def clean_sync(inst):
        si = inst.sync_info
        if si is None:
            return True
        return not getattr(si, "on_wait", None) and \
            not getattr(si, "on_update", None)

    n_removed = 0

    def walk(blocks):
        nonlocal n_removed
        for blk in blocks:
            insts = getattr(blk, "instructions", None)
            if insts:
                last = None
                keep = []
                for inst in insts:
                    op = inst.opcode
                    if op == "Ldweights":
                        k = key(inst)
                        if last == k and clean_sync(inst):
                            n_removed += 1
                            continue
                        last = k
                    elif op not in ("Matmult", "EventSemaphore"):
                        # any other instruction on this block: be safe and
                        # only reset tracking if it could touch PE state
                        if op not in ("TensorScalarPtr", "TensorTensor",
                                      "Activation", "TensorCopy", "Memset",
                                      "DMACopy", "TensorScalarAffineSelect",
                                      "ISA", "RegisterMove"):
                            last = None
                    keep.append(inst)
                if len(keep) != len(insts):
                    blk.set_instructions_from_list(keep) if hasattr(
                        blk, "set_instructions_from_list") else None
                    if not hasattr(blk, "set_instructions_from_list"):
                        del insts[:]
                        insts.extend(keep)
            walk(getattr(blk, "blocks", []) or [])

    walk(nc.m.functions[0].blocks)
    return n_removed


def _prepare_in_maps(x, W_attn, b_attn):
    x = np.asarray(x, dtype=np.float32)
    W = np.asarray(W_attn, dtype=np.float32)
    bb = np.asarray(b_attn, dtype=np.float32)
    SC = np.float32(1.0 / np.sqrt(D))

    xT16 = [np.ascontiguousarray(x[b].T).astype(np.float16) for b in range(B)]

    in_maps = []
    for core in range(NCORES):
        b, g = divmod(core, NCORES // B)
        H = [g * HPC + h for h in range(HPC)]
        q_rows = [W[h * D:(h + 1) * D] for h in H]
        k_rows = [W[C + h * D:C + (h + 1) * D] for h in H]
        v_rows = [W[2 * C + h * D:2 * C + (h + 1) * D] for h in H]
        bq = [bb[h * D:(h + 1) * D] for h in H]
        bk = [bb[C + h * D:C + (h + 1) * D] for h in H]
        bv = [bb[2 * C + h * D:2 * C + (h + 1) * D] for h in H]

        # f-tiles: 0 = [q0; q1], 1 = [k0; k1], 2 = [k2; q2]
        wqk_rows = np.concatenate(
            [q_rows[0], q_rows[1], k_rows[0], k_rows[1], k_rows[2], q_rows[2]], 0)
        wqk16 = np.ascontiguousarray(wqk_rows.T).astype(np.float16)   # [768, 384]
        wv16 = np.ascontiguousarray(
            np.concatenate(v_rows, 0).T).astype(np.float16)           # [768, 192]

        bias_qk = np.stack([
            np.concatenate([bq[0], bq[1]]) * SC,
            np.concatenate([bk[0], bk[1]]),
            np.concatenate([bk[2], bq[2] * SC]),
        ]).astype(np.float32)                                          # [3, 128]
        scale_qk = np.stack([
            np.full(128, SC), np.ones(128),
            np.concatenate([np.ones(64), np.full(64, SC)]),
        ]).astype(np.float32)
        bias_v = np.tile(np.concatenate(bv), (128, 1)).astype(np.float32)

        in_maps.append({
            "xT": xT16[b], "wqk": wqk16, "wv": wv16,
            "bias_qk": bias_qk, "scale_qk": scale_qk, "bias_v": bias_v,
        })
    return in_maps


_NC_CACHE = {}


def _get_nc(reps=1, stage=4):
    key = (reps, stage)
    if key not in _NC_CACHE:
        _NC_CACHE[key] = _build(reps, stage)
    return _NC_CACHE[key]


def kernel(x, W_attn, b_attn):
    nc = _get_nc(1)
    in_maps = _prepare_in_maps(x, W_attn, b_attn)
    res = run_bass_kernel_spmd(nc, in_maps, list(range(NCORES)), trace=False)
    y = np.empty((B, T, C), dtype=np.float32)
    for core in range(NCORES):
        b, g = divmod(core, NCORES // B)
        yt = res.results[core]["yt"]          # [3, 64, 2048]
        for h in range(HPC):
            y[b, :, (g * HPC + h) * D:(g * HPC + h + 1) * D] = yt[h].T
    return y


# revision 38
# speedup vs baseline: 1.0776x; 1.0776x over previous
"""Trainium2 Bass kernel for causal ReLU attention (no softmax).

  qkv = x @ W.T + b;  per head: s = (q k^T) * 1/sqrt(64)
  p = relu(causal(s));  y = p @ v

Sharding: 8 cores = 2 batches x 4 head-groups (3 heads each). Each core:
  - qk-projection computed transposed (features on partitions) so q/k land
    as qT/kT [64, T] ready to be matmul operands with d on partitions
  - v-projection computed natural [T, 192]
  - scores: K=128 zero-padded contraction (two heads share a 128-partition
    tile; lhsT = [kT_h; 0] makes each head's matmul full-width)
  - block-causal: fully-masked key blocks skipped, diagonal blocks get a
    restricted column range + triangle mask via one fused DVE/Pool op
  - two chains (A, B) software-pipelined on the PE queue with one-step
    lookahead so the PSUM->SBUF relu never stalls the PE; relu work is
    spread across DVE (A), ACT (B full) and Pool (B diag); y PSUM
    evacuation runs on Pool.
All matmul operands fp16 (fp32 PSUM accumulation). Host does the
shard/transpose/cast prep and the final gather (pure numpy).
"""
import numpy as np

import concourse.bass as bass
import concourse.mybir as mybir
import concourse.tile as tile
from concourse import bacc
from concourse.bass_utils import run_bass_kernel_spmd

F32 = mybir.dt.float32
F16 = mybir.dt.float16

B, T, C = 2, 2048, 768
NH = 12          # total heads
HPC = 3          # heads per core
D = 64
NCORES = 8
CC = 6           # contraction chunks (768 / 128)
TB = 512         # query block
KB = 128         # key block
NTB = T // TB    # 4
NKB = T // KB    # 16

Relu = mybir.ActivationFunctionType.Relu
Copy = mybir.ActivationFunctionType.Identity
MAX = mybir.AluOpType.max
MULT = mybir.AluOpType.mult


def _build(reps=1, stage=4):
    nc = bacc.Bacc(None, target_bir_lowering=False, debug=False)
    xT = nc.declare_dram_parameter("xT", [C, T], F16, isOutput=False)
    wqk = nc.declare_dram_parameter("wqk", [C, 384], F16, isOutput=False)
    wv = nc.declare_dram_parameter("wv", [C, 192], F16, isOutput=False)
    bias_qk = nc.declare_dram_parameter("bias_qk", [3, 128], F32, isOutput=False)
    scale_qk = nc.declare_dram_parameter("scale_qk", [3, 128], F32, isOutput=False)
    bias_v = nc.declare_dram_parameter("bias_v", [128, 192], F32, isOutput=False)
    yt_out = nc.declare_dram_parameter("yt", [HPC, D, T], F32, isOutput=True)

    with tile.TileContext(nc) as tc:
        with tc.tile_pool(name="const", bufs=1) as const, \
             tc.tile_pool(name="xr", bufs=12) as xr, \
             tc.tile_pool(name="vt", bufs=16) as vtp, \
             tc.tile_pool(name="pt", bufs=8) as ptp, \
             tc.tile_pool(name="ys", bufs=4) as ysp, \
             tc.tile_pool(name="psmix", bufs=2, space="PSUM") as psmix, \
             tc.tile_pool(name="pssc", bufs=3, space="PSUM") as pssc:

            # ---------------- constants ----------------
            bias_sb = const.tile([128, 3], F32)
            scale_sb = const.tile([128, 3], F32)
            nc.sync.dma_start(out=bias_sb, in_=bias_qk[:, :].rearrange("a p -> p a"))
            nc.sync.dma_start(out=scale_sb, in_=scale_qk[:, :].rearrange("a p -> p a"))
            biasv_sb = const.tile([128, 192], F32)
            nc.sync.dma_start(out=biasv_sb, in_=bias_v[:, :])
            # triangle mask M[kk, qq] = 1 if qq >= kk else 0  (f32: faster DVE read)
            mask_sb = const.tile([128, TB], F32)
            nc.vector.memset(mask_sb, 1.0)
            nc.gpsimd.affine_select(
                out=mask_sb, in_=mask_sb,
                compare_op=mybir.AluOpType.is_ge, fill=0.0, base=0,
                pattern=[[1, TB]], channel_multiplier=-1)

            # weight chunks
            wqk_sb = [const.tile([128, 384], F16, tag="wqk", bufs=CC, name=f"wqk{c}") for c in range(CC)]
            wv_sb = [const.tile([128, 192], F16, tag="wv", bufs=CC, name=f"wv{c}") for c in range(CC)]
            for c in range(CC):
                nc.sync.dma_start(out=wqk_sb[c], in_=wqk[c * 128:(c + 1) * 128, :])
                nc.sync.dma_start(out=wv_sb[c], in_=wv[c * 128:(c + 1) * 128, :])

            # persistent attention operand tiles (scores use K=64 contraction
            # with matching base partitions -- no zero padding needed)
            qq01 = const.tile([128, T], F16)   # [qT_h0; qT_h1]
            kAB = const.tile([128, T], F16)    # [kT_h0; kT_h1]
            m2 = const.tile([128, T], F16)     # [kT_h2; qT_h2]
            q2lo = const.tile([64, T], F16)    # qT_h2 shifted to partitions 0-63

            def body():
                # stage: 1=dma only, 2=+proj, 3=+scores/relu, 4=full
                # ---------------- load xT (fp16, pre-cast on host) ---------
                xt = [xr.tile([128, T], F16, tag="xt", name=f"xt{c}") for c in range(CC)]
                for c in range(CC):
                    nc.sync.dma_start(out=xt[c], in_=xT[c * 128:(c + 1) * 128, :])

                if stage < 2:
                    return
                # ---------------- qk projection (transposed) ---------------
                # f-tiles: 0 = [q0; q1], 1 = [k0; k1], 2 = [k2; q2]
                # c-outer order: all 4 query blocks accumulate in parallel
                # (two 2-bank PSUM tiles), so the 4 matmuls sharing one
                # weight chunk are consecutive -> redundant Ldweights are
                # stripped post-finalize by _dedup_ldweights.
                for ft in range(3):
                    pst = [pssc.tile([128, 2 * TB], F32, tag="s",
                                     name=f"pj{ft}_{i}") for i in range(2)]
                    for c in range(CC):
                        for tb in range(NTB):
                            nc.tensor.matmul(
                                pst[tb // 2][:, (tb % 2) * TB:(tb % 2 + 1) * TB],
                                wqk_sb[c][:, ft * 128:(ft + 1) * 128],
                                xt[c][:, tb * TB:(tb + 1) * TB],
                                start=(c == 0), stop=(c == CC - 1))
                    dst = (qq01, kAB, m2)[ft]
                    for i in range(2):
                        nc.scalar.activation(
                            dst[:, i * 2 * TB:(i + 1) * 2 * TB], pst[i], Copy,
                            bias=bias_sb[:, ft:ft + 1],
                            scale=scale_sb[:, ft:ft + 1])
                # shift qT_h2 to partitions 0-63 (SBUF->SBUF DMA)
                nc.sync.dma_start(out=q2lo[:, :], in_=m2[64:128, :])

                # ---------------- v projection (natural layout) ------------
                v_sb = []
                for tt in range(NKB):
                    ps = psmix.tile([128, 192], F32, tag="m")
                    for c in range(CC):
                        nc.tensor.matmul(
                            ps, xt[c][:, tt * 128:(tt + 1) * 128], wv_sb[c],
                            start=(c == 0), stop=(c == CC - 1))
                    vt = vtp.tile([128, 192], F16, tag="v")
                    nc.vector.tensor_add(vt, ps, biasv_sb)
                    v_sb.append(vt)

                if stage < 3:
                    return
                # ---------------- attention ----------------
                # per head: (kT rows, qT rows), both 64-partition slices with
                # matching base partitions (K=64 contraction)
                heads = [(kAB[0:64, :], qq01[0:64, :]),
                         (kAB[64:128, :], qq01[64:128, :]),
                         (m2[0:64, :], q2lo[:, :])]

                def relu_op(eng, out, in_, masked, n):
                    # out = relu(in_) [* mask] on the given engine.
                    # Pool/GPSIMD cannot read PSUM, so masked relus run either
                    # fully on DVE (fused max*mask) or as ACT relu followed by
                    # an in-place Pool affine_select on the SBUF result.
                    if masked:
                        if eng == "dve":
                            nc.vector.scalar_tensor_tensor(
                                out=out, in0=in_, scalar=0.0,
                                in1=mask_sb[:, 0:n], op0=MAX, op1=MULT)
                        else:
                            nc.scalar.activation(out, in_, Relu)
                            nc.gpsimd.affine_select(
                                out=out, in_=out,
                                compare_op=mybir.AluOpType.is_ge, fill=0.0,
                                base=0, pattern=[[1, n]],
                                channel_multiplier=-1)
                    elif eng == "act":
                        nc.scalar.activation(out, in_, Relu)
                    else:
                        nc.vector.tensor_scalar(out, in_, 0.0, None, MAX)

                def make_units(hl, qb, ytp_f, ytp_tp, side):
                    """Build the unit list for one (head, query-block) chain.

                    Each unit is (emit_s, emit_relu(eng), emit_y) closures over
                    a pair of key blocks; tiles are allocated at emission time.
                    """
                    kz, qq = heads[hl]
                    nkb = 4 * qb + 4
                    nfull = 4 * qb
                    uid = f"{side}{hl}_{qb}"
                    units = []
                    box = {}

                    def mk_full(kp):
                        def emit_s(h2):
                            if h2 == 0:
                                box[("s", kp)] = pssc.tile(
                                    [128, 2 * TB], F32, tag="s",
                                    name=f"sp{uid}_{kp}")
                            sp2 = box[("s", kp)]
                            kb = 2 * kp + h2
                            nc.tensor.matmul(
                                sp2[:, h2 * TB:(h2 + 1) * TB],
                                kz[:, kb * KB:(kb + 1) * KB],
                                qq[:, qb * TB:(qb + 1) * TB],
                                start=True, stop=True)

                        def emit_relu(eng):
                            sp2 = box.pop(("s", kp))
                            pt2 = ptp.tile([128, 2 * TB], F16, tag="p2",
                                           name=f"pt{uid}_{kp}")
                            box[("p", kp)] = pt2
                            relu_op(eng, pt2, sp2, False, 0)

                        def emit_y():
                            pt2 = box.pop(("p", kp))
                            if stage >= 4:
                                for h2 in range(2):
                                    kb = 2 * kp + h2
                                    nc.tensor.matmul(
                                        ytp_f(),
                                        v_sb[kb][:, hl * 64:(hl + 1) * 64],
                                        pt2[:, h2 * TB:(h2 + 1) * TB],
                                        start=(kb == 0), stop=False,
                                        tile_position=ytp_tp)
                        return (emit_s, emit_relu, emit_y)

                    def mk_diag(dp):
                        def emit_s(h2):
                            if h2 == 0:
                                box[("sd", dp)] = pssc.tile(
                                    [128, 2 * TB], F32, tag="s",
                                    name=f"spd{uid}_{dp}")
                            sp2 = box[("sd", dp)]
                            j = 2 * dp + h2
                            kb = nfull + j
                            lo = j * KB
                            off = h2 * TB
                            nc.tensor.matmul(
                                sp2[:, off + lo:off + TB],
                                kz[:, kb * KB:(kb + 1) * KB],
                                qq[:, qb * TB + lo:(qb + 1) * TB],
                                start=True, stop=True)

                        def emit_relu(eng):
                            # split the two sub-blocks between the DVE path
                            # and the ACT+Pool path, opposite order per side,
                            # so each engine gets one big and one small block.
                            sp2 = box.pop(("sd", dp))
                            pt2 = ptp.tile([128, 2 * TB], F16, tag="p2",
                                           name=f"ptd{uid}_{dp}")
                            box[("pd", dp)] = pt2
                            pick = (("dve", "actpool") if side[0] == "a"
                                    else ("actpool", "dve"))
                            for h2 in range(2):
                                j = 2 * dp + h2
                                lo = j * KB
                                n = TB - lo
                                off = h2 * TB
                                relu_op(pick[h2], pt2[:, off + lo:off + TB],
                                        sp2[:, off + lo:off + TB], True, n)

                        def emit_y():
                            pt2 = box.pop(("pd", dp))
                            if stage >= 4:
                                for h2 in range(2):
                                    j = 2 * dp + h2
                                    kb = nfull + j
                                    lo = j * KB
                                    off = h2 * TB
                                    nc.tensor.matmul(
                                        ytp_f()[:, lo:TB],
                                        v_sb[kb][:, hl * 64:(hl + 1) * 64],
                                        pt2[:, off + lo:off + TB],
                                        start=(kb == 0), stop=(kb == nkb - 1),
                                        tile_position=ytp_tp)
                        return (emit_s, emit_relu, emit_y)

                    for kp in range(nfull // 2):
                        units.append((mk_full(kp), False))
                    for dp in range(2):
                        units.append((mk_diag(dp), True))

                    def emit_out():
                        if stage >= 4:
                            ys = ysp.tile([64, TB], F32, tag="ys",
                                          name=f"ys{uid}")
                            nc.scalar.activation(ys, ytp_f(), Copy)
                            nc.sync.dma_start(
                                out=yt_out[hl, :, qb * TB:(qb + 1) * TB],
                                in_=ys)
                    return units, emit_out

                # Build one global, software-pipelined stream of steps across
                # all chain pairs: lookahead-2 on the PE queue means the relu
                # of step k overlaps the score matmuls of steps k+1 and k+2,
                # and there is no pipeline reset at pair boundaries. Pair
                # order keeps every pair between two users of the same PSUM
                # buf long enough to hide the y evacuation (psmix bufs=2).
                # same-head pairs: at each step both sides hit the same key
                # block, so h2-interleaved s-emission makes the second score
                # Ldweights a strippable duplicate
                pairs = [((0, 0), (0, 1)), ((1, 0), (1, 1)), ((2, 0), (2, 1)),
                         ((0, 2), (0, 3)), ((1, 2), (1, 3)), ((2, 2), (2, 3))]
                steps = []
                ytp_cache = {}

                def mk_ytp(pi, sl):
                    def f():
                        if pi not in ytp_cache:
                            ytp_cache[pi] = psmix.tile(
                                [128, TB], F32, tag="m", name=f"yt2_{pi}")
                        return ytp_cache[pi][sl]
                    return f

                for pi, (ca, cb) in enumerate(pairs):
                    usA, outA = make_units(ca[0], ca[1],
                                           mk_ytp(pi, slice(0, 64)), (0, 0),
                                           f"a{pi}")
                    usB, outB = make_units(cb[0], cb[1],
                                           mk_ytp(pi, slice(64, 128)),
                                           (0, 64), f"b{pi}")
                    nA, nB = len(usA), len(usB)
                    for k in range(max(nA, nB)):
                        st = {}
                        for side, us, out, cn in (("a", usA, outA, nA),
                                                  ("b", usB, outB, nB)):
                            if k < len(us):
                                (es, er, ey), diag = us[k]
                                st[side] = (es, er, ey, diag,
                                            out if k == cn - 1 else None)
                        steps.append(st)

                def emit(i, phase):
                    # engine choice by global step parity: A and B always land
                    # on different engines; diagonal (masked) relus never go
                    # to ACT (it cannot apply the mask in one op).
                    st = steps[i]
                    if phase == 0:
                        # h2-interleaved score emission: the two sides' kb
                        # matmuls share the kz weight load (dedup strips the
                        # duplicate Ldweights)
                        for h2 in range(2):
                            for side in ("a", "b"):
                                if side in st:
                                    st[side][0](h2)
                        return
                    for side in ("a", "b"):
                        if side not in st:
                            continue
                        es, er, ey, diag, out = st[side]
                        if phase == 0:
                            pass  # handled at step level (h2-interleaved)
                        elif phase == 1:
                            if side == "a":
                                eng = ("dve", "act")[i % 2]
                            else:
                                eng = ("act", "dve")[i % 2]
                            er(eng)  # diag units pick their own engines
                        else:
                            ey()
                            if out is not None:
                                out()

                n = len(steps)
                emit(0, 0)
                if n > 1:
                    emit(1, 0)
                for k in range(n):
                    if k + 2 < n:
                        emit(k + 2, 0)     # lookahead scores
                    emit(k, 1)             # relu of step k
                    emit(k, 2)             # y of step k (+chain output)

            if reps == 1:
                body()
            elif reps < 0:
                with tc.For_i(0, -reps, 1):
                    body()
            else:
                for _ in range(reps):
                    body()

    nc.finalize()
    _dedup_ldweights(nc)
    return nc


def _dedup_ldweights(nc):
    """Drop Ldweights that reload the exact weights already resident.

    The legalizer pairs every Matmult with its own Ldweights; matmuls
    emitted back-to-back with the same stationary operand reload it
    needlessly (~53-106ns each on HW). Safe removal requires: identical
    weights AP and tile config, nothing but matmuls/event-sems between
    the two loads, and no semaphore waits/updates on the removed load.
    """
    def key(inst):
        return (repr(inst.ins[0]), repr(inst.tile_size),
                repr(inst.tile_position), repr(inst.perf_mode),
                repr(inst.is_transpose))

    def clean_sync(inst):
        si = inst.sync_info
        if si is None:
            return True
        return not getattr(si, "on_wait", None) and \
            not getattr(si, "on_update", None)

    n_removed = 0

    def walk(blocks):
        nonlocal n_removed
        for blk in blocks:
            insts = getattr(blk, "instructions", None)
            if insts:
                last = None
                keep = []
                for inst in insts:
                    op = inst.opcode
                    if op == "Ldweights":
                        k = key(inst)
                        if last == k and clean_sync(inst):
                            n_removed += 1
                            continue
                        last = k
                    elif op not in ("Matmult", "EventSemaphore"):
                        # any other instruction on this block: be safe and
                        # only reset tracking if it could touch PE state
                        if op not in ("TensorScalarPtr", "TensorTensor",
                                      "Activation", "TensorCopy", "Memset",
                                      "DMACopy", "TensorScalarAffineSelect",
                                      "ISA", "RegisterMove"):
                            last = None
                    keep.append(inst)
                if len(keep) != len(insts):
                    blk.set_instructions_from_list(keep) if hasattr(
                        blk, "set_instructions_from_list") else None
                    if not hasattr(blk, "set_instructions_from_list"):
                        del insts[:]
                        insts.extend(keep)
            walk(getattr(blk, "blocks", []) or [])

    walk(nc.m.functions[0].blocks)
    return n_removed


def _prepare_in_maps(x, W_attn, b_attn):
    x = np.asarray(x, dtype=np.float32)
    W = np.asarray(W_attn, dtype=np.float32)
    bb = np.asarray(b_attn, dtype=np.float32)
    SC = np.float32(1.0 / np.sqrt(D))

    xT16 = [np.ascontiguousarray(x[b].T).astype(np.float16) for b in range(B)]

    in_maps = []
    for core in range(NCORES):
        b, g = divmod(core, NCORES // B)
        H = [g * HPC + h for h in range(HPC)]
        q_rows = [W[h * D:(h + 1) * D] for h in H]
        k_rows = [W[C + h * D:C + (h + 1) * D] for h in H]
        v_rows = [W[2 * C + h * D:2 * C + (h + 1) * D] for h in H]
        bq = [bb[h * D:(h + 1) * D] for h in H]
        bk = [bb[C + h * D:C + (h + 1) * D] for h in H]
        bv = [bb[2 * C + h * D:2 * C + (h + 1) * D] for h in H]

        # f-tiles: 0 = [q0; q1], 1 = [k0; k1], 2 = [k2; q2]
        wqk_rows = np.concatenate(
            [q_rows[0], q_rows[1], k_rows[0], k_rows[1], k_rows[2], q_rows[2]], 0)
        wqk16 = np.ascontiguousarray(wqk_rows.T).astype(np.float16)   # [768, 384]
        wv16 = np.ascontiguousarray(
            np.concatenate(v_rows, 0).T).astype(np.float16)           # [768, 192]

        bias_qk = np.stack([
            np.concatenate([bq[0], bq[1]]) * SC,
            np.concatenate([bk[0], bk[1]]),
            np.concatenate([bk[2], bq[2] * SC]),
        ]).astype(np.float32)                                          # [3, 128]
        scale_qk = np.stack([
            np.full(128, SC), np.ones(128),
            np.concatenate([np.ones(64), np.full(64, SC)]),
        ]).astype(np.float32)
        bias_v = np.tile(np.concatenate(bv), (128, 1)).astype(np.float32)

        in_maps.append({
            "xT": xT16[b], "wqk": wqk16, "wv": wv16,
            "bias_qk": bias_qk, "scale_qk": scale_qk, "bias_v": bias_v,
        })
    return in_maps


_NC_CACHE = {}


def _get_nc(reps=1, stage=4):
    key = (reps, stage)
    if key not in _NC_CACHE:
        _NC_CACHE[key] = _build(reps, stage)
    return _NC_CACHE[key]


def kernel(x, W_attn, b_attn):
    nc = _get_nc(1)
    in_maps = _prepare_in_maps(x, W_attn, b_attn)
    res = run_bass_kernel_spmd(nc, in_maps, list(range(NCORES)), trace=False)
    y = np.empty((B, T, C), dtype=np.float32)
    for core in range(NCORES):
        b, g = divmod(core, NCORES // B)
        yt = res.results[core]["yt"]          # [3, 64, 2048]
        for h in range(HPC):
            y[b, :, (g * HPC + h) * D:(g * HPC + h + 1) * D] = yt[h].T
    return y


# revision 40
# speedup vs baseline: 1.1109x; 1.0308x over previous
"""Trainium2 Bass kernel for causal ReLU attention (no softmax).

  qkv = x @ W.T + b;  per head: s = (q k^T) * 1/sqrt(64)
  p = relu(causal(s));  y = p @ v

Sharding: 8 cores = 2 batches x 4 head-groups (3 heads each). Each core:
  - qk-projection computed transposed (features on partitions) so q/k land
    as qT/kT [64, T] ready to be matmul operands with d on partitions
  - v-projection computed natural [T, 192]
  - scores: K=128 zero-padded contraction (two heads share a 128-partition
    tile; lhsT = [kT_h; 0] makes each head's matmul full-width)
  - block-causal: fully-masked key blocks skipped, diagonal blocks get a
    restricted column range + triangle mask via one fused DVE/Pool op
  - two chains (A, B) software-pipelined on the PE queue with one-step
    lookahead so the PSUM->SBUF relu never stalls the PE; relu work is
    spread across DVE (A), ACT (B full) and Pool (B diag); y PSUM
    evacuation runs on Pool.
All matmul operands fp16 (fp32 PSUM accumulation). Host does the
shard/transpose/cast prep and the final gather (pure numpy).
"""
import numpy as np

import concourse.bass as bass
import concourse.mybir as mybir
import concourse.tile as tile
from concourse import bacc
from concourse.bass_utils import run_bass_kernel_spmd

F32 = mybir.dt.float32
F16 = mybir.dt.float16

B, T, C = 2, 2048, 768
NH = 12          # total heads
HPC = 3          # heads per core
D = 64
NCORES = 8
CC = 6           # contraction chunks (768 / 128)
TB = 512         # query block
KB = 128         # key block
NTB = T // TB    # 4
NKB = T // KB    # 16

Relu = mybir.ActivationFunctionType.Relu
Copy = mybir.ActivationFunctionType.Identity
MAX = mybir.AluOpType.max
MULT = mybir.AluOpType.mult


def _build(reps=1, stage=4):
    nc = bacc.Bacc(None, target_bir_lowering=False, debug=False)
    xT = nc.declare_dram_parameter("xT", [C, T], F16, isOutput=False)
    wqk = nc.declare_dram_parameter("wqk", [C, 384], F16, isOutput=False)
    wv = nc.declare_dram_parameter("wv", [C, 192], F16, isOutput=False)
    bias_qk = nc.declare_dram_parameter("bias_qk", [3, 128], F32, isOutput=False)
    scale_qk = nc.declare_dram_parameter("scale_qk", [3, 128], F32, isOutput=False)
    bias_v = nc.declare_dram_parameter("bias_v", [128, 192], F32, isOutput=False)
    yt_out = nc.declare_dram_parameter("yt", [HPC, D, T], F32, isOutput=True)

    with tile.TileContext(nc) as tc:
        with tc.tile_pool(name="const", bufs=1) as const, \
             tc.tile_pool(name="xr", bufs=12) as xr, \
             tc.tile_pool(name="vt", bufs=16) as vtp, \
             tc.tile_pool(name="pt", bufs=8) as ptp, \
             tc.tile_pool(name="ys", bufs=4) as ysp, \
             tc.tile_pool(name="psmix", bufs=2, space="PSUM") as psmix, \
             tc.tile_pool(name="pssc", bufs=3, space="PSUM") as pssc:

            # ---------------- constants ----------------
            bias_sb = const.tile([128, 3], F32)
            scale_sb = const.tile([128, 3], F32)
            nc.sync.dma_start(out=bias_sb, in_=bias_qk[:, :].rearrange("a p -> p a"))
            nc.sync.dma_start(out=scale_sb, in_=scale_qk[:, :].rearrange("a p -> p a"))
            biasv_sb = const.tile([128, 192], F32)
            nc.sync.dma_start(out=biasv_sb, in_=bias_v[:, :])
            # triangle mask M[kk, qq] = 1 if qq >= kk else 0  (f32: faster DVE read)
            mask_sb = const.tile([128, TB], F32)
            nc.vector.memset(mask_sb, 1.0)
            nc.gpsimd.affine_select(
                out=mask_sb, in_=mask_sb,
                compare_op=mybir.AluOpType.is_ge, fill=0.0, base=0,
                pattern=[[1, TB]], channel_multiplier=-1)

            # weight chunks
            wqk_sb = [const.tile([128, 384], F16, tag="wqk", bufs=CC, name=f"wqk{c}") for c in range(CC)]
            wv_sb = [const.tile([128, 192], F16, tag="wv", bufs=CC, name=f"wv{c}") for c in range(CC)]
            for c in range(CC):
                nc.sync.dma_start(out=wqk_sb[c], in_=wqk[c * 128:(c + 1) * 128, :])
                nc.sync.dma_start(out=wv_sb[c], in_=wv[c * 128:(c + 1) * 128, :])

            # persistent attention operand tiles (scores use K=64 contraction
            # with matching base partitions -- no zero padding needed)
            qq01 = const.tile([128, T], F16)   # [qT_h0; qT_h1]
            kAB = const.tile([128, T], F16)    # [kT_h0; kT_h1]
            m2 = const.tile([128, T], F16)     # [kT_h2; qT_h2]
            q2lo = const.tile([64, T], F16)    # qT_h2 shifted to partitions 0-63

            def body():
                # stage: 1=dma only, 2=+proj, 3=+scores/relu, 4=full
                # ---------------- load xT (fp16, pre-cast on host) ---------
                xt = [xr.tile([128, T], F16, tag="xt", name=f"xt{c}") for c in range(CC)]
                for c in range(CC):
                    nc.sync.dma_start(out=xt[c], in_=xT[c * 128:(c + 1) * 128, :])

                if stage < 2:
                    return
                # ---------------- qk projection (transposed) ---------------
                # f-tiles: 0 = [q0; q1], 1 = [k0; k1], 2 = [k2; q2]
                # c-outer order: all 4 query blocks accumulate in parallel
                # (two 2-bank PSUM tiles), so the 4 matmuls sharing one
                # weight chunk are consecutive -> redundant Ldweights are
                # stripped post-finalize by _dedup_ldweights.
                for ft in range(3):
                    pst = [pssc.tile([128, 2 * TB], F32, tag="s",
                                     name=f"pj{ft}_{i}") for i in range(2)]
                    for c in range(CC):
                        for tb in range(NTB):
                            nc.tensor.matmul(
                                pst[tb // 2][:, (tb % 2) * TB:(tb % 2 + 1) * TB],
                                wqk_sb[c][:, ft * 128:(ft + 1) * 128],
                                xt[c][:, tb * TB:(tb + 1) * TB],
                                start=(c == 0), stop=(c == CC - 1))
                    dst = (qq01, kAB, m2)[ft]
                    for i in range(2):
                        nc.scalar.activation(
                            dst[:, i * 2 * TB:(i + 1) * 2 * TB], pst[i], Copy,
                            bias=bias_sb[:, ft:ft + 1],
                            scale=scale_sb[:, ft:ft + 1])
                # shift qT_h2 to partitions 0-63 (SBUF->SBUF DMA)
                nc.sync.dma_start(out=q2lo[:, :], in_=m2[64:128, :])

                # ---------------- v projection (natural layout) ------------
                v_sb = []
                for tt in range(NKB):
                    ps = psmix.tile([128, 192], F32, tag="m")
                    for c in range(CC):
                        nc.tensor.matmul(
                            ps, xt[c][:, tt * 128:(tt + 1) * 128], wv_sb[c],
                            start=(c == 0), stop=(c == CC - 1))
                    vt = vtp.tile([128, 192], F16, tag="v")
                    nc.vector.tensor_add(vt, ps, biasv_sb)
                    v_sb.append(vt)

                if stage < 3:
                    return
                # ---------------- attention ----------------
                # per head: (kT rows, qT rows), both 64-partition slices with
                # matching base partitions (K=64 contraction)
                heads = [(kAB[0:64, :], qq01[0:64, :]),
                         (kAB[64:128, :], qq01[64:128, :]),
                         (m2[0:64, :], q2lo[:, :])]

                def relu_op(eng, out, in_, masked, n):
                    # out = relu(in_) [* mask] on the given engine.
                    # Pool/GPSIMD cannot read PSUM, so masked relus run either
                    # fully on DVE (fused max*mask) or as ACT relu followed by
                    # an in-place Pool affine_select on the SBUF result.
                    if masked:
                        if eng == "dve":
                            nc.vector.scalar_tensor_tensor(
                                out=out, in0=in_, scalar=0.0,
                                in1=mask_sb[:, 0:n], op0=MAX, op1=MULT)
                        else:
                            nc.scalar.activation(out, in_, Relu)
                            nc.gpsimd.affine_select(
                                out=out, in_=out,
                                compare_op=mybir.AluOpType.is_ge, fill=0.0,
                                base=0, pattern=[[1, n]],
                                channel_multiplier=-1)
                    elif eng == "act":
                        nc.scalar.activation(out, in_, Relu)
                    else:
                        nc.vector.tensor_scalar(out, in_, 0.0, None, MAX)

                def make_units(hl, qb, ytp_f, ytp_tp, side):
                    """Build the unit list for one (head, query-block) chain.

                    Each unit is (emit_s, emit_relu(eng), emit_y) closures over
                    a pair of key blocks; tiles are allocated at emission time.
                    """
                    kz, qq = heads[hl]
                    nkb = 4 * qb + 4
                    nfull = 4 * qb
                    uid = f"{side}{hl}_{qb}"
                    units = []
                    box = {}

                    def mk_full(kp):
                        def emit_s(h2):
                            if h2 == 0:
                                box[("s", kp)] = pssc.tile(
                                    [128, 2 * TB], F32, tag="s",
                                    name=f"sp{uid}_{kp}")
                            sp2 = box[("s", kp)]
                            kb = 2 * kp + h2
                            nc.tensor.matmul(
                                sp2[:, h2 * TB:(h2 + 1) * TB],
                                kz[:, kb * KB:(kb + 1) * KB],
                                qq[:, qb * TB:(qb + 1) * TB],
                                start=True, stop=True)

                        def emit_relu(eng):
                            sp2 = box.pop(("s", kp))
                            pt2 = ptp.tile([128, 2 * TB], F16, tag="p2",
                                           name=f"pt{uid}_{kp}")
                            box[("p", kp)] = pt2
                            relu_op(eng, pt2, sp2, False, 0)

                        def emit_y():
                            pt2 = box.pop(("p", kp))
                            if stage >= 4:
                                for h2 in range(2):
                                    kb = 2 * kp + h2
                                    nc.tensor.matmul(
                                        ytp_f(),
                                        v_sb[kb][:, hl * 64:(hl + 1) * 64],
                                        pt2[:, h2 * TB:(h2 + 1) * TB],
                                        start=(kb == 0), stop=False,
                                        tile_position=ytp_tp)
                        return (emit_s, emit_relu, emit_y)

                    def mk_diag(dp):
                        def emit_s(h2):
                            if h2 == 0:
                                box[("sd", dp)] = pssc.tile(
                                    [128, 2 * TB], F32, tag="s",
                                    name=f"spd{uid}_{dp}")
                            sp2 = box[("sd", dp)]
                            j = 2 * dp + h2
                            kb = nfull + j
                            lo = j * KB
                            off = h2 * TB
                            nc.tensor.matmul(
                                sp2[:, off + lo:off + TB],
                                kz[:, kb * KB:(kb + 1) * KB],
                                qq[:, qb * TB + lo:(qb + 1) * TB],
                                start=True, stop=True)

                        def emit_relu(eng):
                            # split the two sub-blocks between the DVE path
                            # and the ACT+Pool path, opposite order per side,
                            # so each engine gets one big and one small block.
                            sp2 = box.pop(("sd", dp))
                            pt2 = ptp.tile([128, 2 * TB], F16, tag="p2",
                                           name=f"ptd{uid}_{dp}")
                            box[("pd", dp)] = pt2
                            pick = (("dve", "actpool") if side[0] == "a"
                                    else ("actpool", "dve"))
                            for h2 in range(2):
                                j = 2 * dp + h2
                                lo = j * KB
                                n = TB - lo
                                off = h2 * TB
                                relu_op(pick[h2], pt2[:, off + lo:off + TB],
                                        sp2[:, off + lo:off + TB], True, n)

                        def emit_y():
                            pt2 = box.pop(("pd", dp))
                            if stage >= 4:
                                for h2 in range(2):
                                    j = 2 * dp + h2
                                    kb = nfull + j
                                    lo = j * KB
                                    off = h2 * TB
                                    nc.tensor.matmul(
                                        ytp_f()[:, lo:TB],
                                        v_sb[kb][:, hl * 64:(hl + 1) * 64],
                                        pt2[:, off + lo:off + TB],
                                        start=(kb == 0), stop=(kb == nkb - 1),
                                        tile_position=ytp_tp)
                        return (emit_s, emit_relu, emit_y)

                    for kp in range(nfull // 2):
                        units.append((mk_full(kp), False))
                    for dp in range(2):
                        units.append((mk_diag(dp), True))

                    def emit_out():
                        if stage >= 4:
                            ys = ysp.tile([64, TB], F32, tag="ys",
                                          name=f"ys{uid}")
                            nc.scalar.activation(ys, ytp_f(), Copy)
                            nc.sync.dma_start(
                                out=yt_out[hl, :, qb * TB:(qb + 1) * TB],
                                in_=ys)
                    return units, emit_out

                # Build one global, software-pipelined stream of steps across
                # all chain pairs: lookahead-2 on the PE queue means the relu
                # of step k overlaps the score matmuls of steps k+1 and k+2,
                # and there is no pipeline reset at pair boundaries. Pair
                # order keeps every pair between two users of the same PSUM
                # buf long enough to hide the y evacuation (psmix bufs=2).
                # same-head pairs: at each step both sides hit the same key
                # block, so h2-interleaved s-emission makes the second score
                # Ldweights a strippable duplicate
                pairs = [((0, 0), (0, 1)), ((1, 0), (1, 1)), ((2, 0), (2, 1)),
                         ((0, 2), (0, 3)), ((1, 2), (1, 3)), ((2, 2), (2, 3))]
                steps = []
                ytp_cache = {}

                def mk_ytp(pi, sl):
                    def f():
                        if pi not in ytp_cache:
                            ytp_cache[pi] = psmix.tile(
                                [128, TB], F32, tag="m", name=f"yt2_{pi}")
                        return ytp_cache[pi][sl]
                    return f

                for pi, (ca, cb) in enumerate(pairs):
                    usA, outA = make_units(ca[0], ca[1],
                                           mk_ytp(pi, slice(0, 64)), (0, 0),
                                           f"a{pi}")
                    usB, outB = make_units(cb[0], cb[1],
                                           mk_ytp(pi, slice(64, 128)),
                                           (0, 64), f"b{pi}")
                    nA, nB = len(usA), len(usB)
                    for k in range(max(nA, nB)):
                        st = {}
                        for side, us, out, cn in (("a", usA, outA, nA),
                                                  ("b", usB, outB, nB)):
                            if k < len(us):
                                (es, er, ey), diag = us[k]
                                st[side] = (es, er, ey, diag,
                                            out if k == cn - 1 else None)
                        steps.append(st)

                def emit(i, phase):
                    # engine choice by global step parity: A and B always land
                    # on different engines; diagonal (masked) relus never go
                    # to ACT (it cannot apply the mask in one op).
                    st = steps[i]
                    if phase == 0:
                        # h2-interleaved score emission: the two sides' kb
                        # matmuls share the kz weight load (dedup strips the
                        # duplicate Ldweights)
                        for h2 in range(2):
                            for side in ("a", "b"):
                                if side in st:
                                    st[side][0](h2)
                        return
                    for side in ("a", "b"):
                        if side not in st:
                            continue
                        es, er, ey, diag, out = st[side]
                        if phase == 0:
                            pass  # handled at step level (h2-interleaved)
                        elif phase == 1:
                            if side == "a":
                                eng = ("dve", "act")[i % 2]
                            else:
                                eng = ("act", "dve")[i % 2]
                            er(eng)  # diag units pick their own engines
                        else:
                            ey()
                            if out is not None:
                                out()

                n = len(steps)
                emit(0, 0)
                if n > 1:
                    emit(1, 0)
                for k in range(n):
                    if k + 2 < n:
                        emit(k + 2, 0)     # lookahead scores
                    emit(k, 1)             # relu of step k
                    emit(k, 2)             # y of step k (+chain output)

            if reps == 1:
                body()
            elif reps < 0:
                # unroll x2 inside the hardware loop: the Tile scheduler works
                # per block, so pairing two bodies lets one iteration's tail
                # (relus, y evac, output DMA) overlap the next one's
                # projection instead of draining at the loop-back edge.
                if (-reps) % 2 == 0:
                    with tc.For_i(0, -reps // 2, 1):
                        body()
                        body()
                else:
                    with tc.For_i(0, -reps, 1):
                        body()
            else:
                for _ in range(reps):
                    body()

    nc.finalize()
    _dedup_ldweights(nc)
    return nc


def _dedup_ldweights(nc):
    """Drop Ldweights that reload the exact weights already resident.

    The legalizer pairs every Matmult with its own Ldweights; matmuls
    emitted back-to-back with the same stationary operand reload it
    needlessly (~53-106ns each on HW). Safe removal requires: identical
    weights AP and tile config, nothing but matmuls/event-sems between
    the two loads, and no semaphore waits/updates on the removed load.
    """
    def key(inst):
        return (repr(inst.ins[0]), repr(inst.tile_size),
                repr(inst.tile_position), repr(inst.perf_mode),
                repr(inst.is_transpose))

    def clean_sync(inst):
        si = inst.sync_info
        if si is None:
            return True
        return not getattr(si, "on_wait", None) and \
            not getattr(si, "on_update", None)

    n_removed = 0

    def walk(blocks):
        nonlocal n_removed
        for blk in blocks:
            insts = getattr(blk, "instructions", None)
            if insts:
                last = None
                keep = []
                for inst in insts:
                    op = inst.opcode
                    if op == "Ldweights":
                        k = key(inst)
                        if last == k and clean_sync(inst):
                            n_removed += 1
                            continue
                        last = k
                    elif op not in ("Matmult", "EventSemaphore"):
                        # any other instruction on this block: be safe and
                        # only reset tracking if it could touch PE state
                        if op not in ("TensorScalarPtr", "TensorTensor",
                                      "Activation", "TensorCopy", "Memset",
                                      "DMACopy", "TensorScalarAffineSelect",
                                      "ISA", "RegisterMove"):
                            last = None
                    keep.append(inst)
                if len(keep) != len(insts):
                    blk.set_instructions_from_list(keep) if hasattr(
                        blk, "set_instructions_from_list") else None
                    if not hasattr(blk, "set_instructions_from_list"):
                        del insts[:]
                        insts.extend(keep)
            walk(getattr(blk, "blocks", []) or [])

    walk(nc.m.functions[0].blocks)
    return n_removed


def _prepare_in_maps(x, W_attn, b_attn):
    x = np.asarray(x, dtype=np.float32)
    W = np.asarray(W_attn, dtype=np.float32)
    bb = np.asarray(b_attn, dtype=np.float32)
    SC = np.float32(1.0 / np.sqrt(D))

    xT16 = [np.ascontiguousarray(x[b].T).astype(np.float16) for b in range(B)]

    in_maps = []
    for core in range(NCORES):
        b, g = divmod(core, NCORES // B)
        H = [g * HPC + h for h in range(HPC)]
        q_rows = [W[h * D:(h + 1) * D] for h in H]
        k_rows = [W[C + h * D:C + (h + 1) * D] for h in H]
        v_rows = [W[2 * C + h * D:2 * C + (h + 1) * D] for h in H]
        bq = [bb[h * D:(h + 1) * D] for h in H]
        bk = [bb[C + h * D:C + (h + 1) * D] for h in H]
        bv = [bb[2 * C + h * D:2 * C + (h + 1) * D] for h in H]

        # f-tiles: 0 = [q0; q1], 1 = [k0; k1], 2 = [k2; q2]
        wqk_rows = np.concatenate(
            [q_rows[0], q_rows[1], k_rows[0], k_rows[1], k_rows[2], q_rows[2]], 0)
        wqk16 = np.ascontiguousarray(wqk_rows.T).astype(np.float16)   # [768, 384]
        wv16 = np.ascontiguousarray(
            np.concatenate(v_rows, 0).T).astype(np.float16)           # [768, 192]

        bias_qk = np.stack([
            np.concatenate([bq[0], bq[1]]) * SC,
            np.concatenate([bk[0], bk[1]]),
            np.concatenate([bk[2], bq[2] * SC]),
        ]).astype(np.float32)                                          # [3, 128]
        scale_qk = np.stack([
            np.full(128, SC), np.ones(128),
            np.concatenate([np.ones(64), np.full(64, SC)]),
        ]).astype(np.float32)
        bias_v = np.tile(np.concatenate(bv), (128, 1)).astype(np.float32)

        in_maps.append({
            "xT": xT16[b], "wqk": wqk16, "wv": wv16,
            "bias_qk": bias_qk, "scale_qk": scale_qk, "bias_v": bias_v,
        })
    return in_maps


_NC_CACHE = {}


def _get_nc(reps=1, stage=4):
    key = (reps, stage)
    if key not in _NC_CACHE:
        _NC_CACHE[key] = _build(reps, stage)
    return _NC_CACHE[key]


def kernel(x, W_attn, b_attn):
    nc = _get_nc(1)
    in_maps = _prepare_in_maps(x, W_attn, b_attn)
    res = run_bass_kernel_spmd(nc, in_maps, list(range(NCORES)), trace=False)
    y = np.empty((B, T, C), dtype=np.float32)
    for core in range(NCORES):
        b, g = divmod(core, NCORES // B)
        yt = res.results[core]["yt"]          # [3, 64, 2048]
        for h in range(HPC):
            y[b, :, (g * HPC + h) * D:(g * HPC + h + 1) * D] = yt[h].T
    return y
